# revision 27
# baseline (speedup 1.0000x reference)
"""Trainium2 Bass kernel for a GPT-2-style transformer block.

Problem: x[4,2048,768] through pre-LN attention (12 heads, causal) + pre-LN MLP
(4x hidden, tanh-approx gelu), residual connections.

Sharding: 8 cores = 4 batch elements x 2-way tensor parallel (heads 0-5 / 6-11
for attention, hidden cols 0-1535 / 1536-3071 for the MLP). Pairwise AllReduce
{0,1}{2,3}{4,5}{6,7} after c_proj and after c_fc2, chunked by token blocks and
ordered so every collective lands before anything in an engine stream waits on
it (in-order engines suffer head-of-line blocking otherwise).

Device layout is feature-major ([C, T]: features on partitions, tokens on the
free dim). The host pre-transposes x, pre-folds LN gains/biases into the weight
matrices, and transposes the output back. LN partition-dim sums are matmuls
against a [128,128] ones matrix, which broadcasts the sums to every partition
for free; all stat math then runs on wide [128,TN] tiles (no single-partition
vector ops anywhere). Softmax uses exp without max subtraction (logits are O(1)
here) with denominators recovered through an appended ones-column on V, a fast
approximate reciprocal, and a partition broadcast applied to the small
attention output. Attention emits each head's S matmuls as one dense block and
the P@V accumulation as a second block so the exp/mask chain never bubbles the
PE. Matmuls run in bf16 (fp32 PSUM accumulation); the residual stream and all
normalization math stay fp32.
"""

import numpy as np
import ml_dtypes

import concourse.bacc as bacc
import concourse.bass as bass
import concourse.mybir as mybir
import concourse.tile as tile
from concourse.bass_utils import run_bass_kernel_spmd

N_CORES = 8
B, T, C = 4, 2048, 768
H = 12
HD = 64
HIDDEN = 4 * C
LN_EPS = 1e-5

NC_CHUNKS = C // 128          # 6 feature chunks
TC = 4                        # token chunks
TN = T // TC                  # 512 tokens per chunk
KT = T // 128                 # 16 k-subtiles
H_LOC = H // 2                # 6 heads per core
QKW = H_LOC * HD              # 384 per-core q/k/v width
HID_LOC = HIDDEN // 2         # 1536 per-core hidden
SCALE = 1.0 / 8.0             # 1/sqrt(64)

F32 = mybir.dt.float32
BF16 = mybir.dt.bfloat16

REPLICA_GROUPS = [[0, 1], [2, 3], [4, 5], [6, 7]]


def _build_nc():
    nc = bacc.Bacc("TRN2", target_bir_lowering=False, debug=False,
                   num_devices=N_CORES)

    x_in = nc.dram_tensor("x_fm", [C, T], F32, kind="ExternalInput")
    wqk = nc.dram_tensor("wqk", [C, 2 * QKW], BF16, kind="ExternalInput")
    wv = nc.dram_tensor("wv", [C, QKW], BF16, kind="ExternalInput")
    wproj = nc.dram_tensor("wproj", [QKW, C], BF16, kind="ExternalInput")
    wfc = nc.dram_tensor("wfc", [C, HID_LOC], BF16, kind="ExternalInput")
    wfc2 = nc.dram_tensor("wfc2", [HID_LOC, C], BF16, kind="ExternalInput")
    bqk_d = nc.dram_tensor("bqk", [128, 6], F32, kind="ExternalInput")
    pbias_d = nc.dram_tensor("pbias", [128, 6], F32, kind="ExternalInput")
    bproj_d = nc.dram_tensor("bproj", [128, 6], F32, kind="ExternalInput")
    bfc_d = nc.dram_tensor("bfc", [128, 12], F32, kind="ExternalInput")
    bfc2_d = nc.dram_tensor("bfc2", [128, 6], F32, kind="ExternalInput")
    out_d = nc.dram_tensor("out_fm", [C, T], F32, kind="ExternalOutput")

    with tile.TileContext(nc) as tc_:
        _emit(nc, tc_, x_in, wqk, wv, wproj, wfc, wfc2,
              bqk_d, pbias_d, bproj_d, bfc_d, bfc2_d, out_d)

    nc.compile()
    return nc


def _ln_sums(nc, pool, psum, x_tiles, ones_m, tcix, tag, pstag, psbufs):
    """Broadcast partition-dim sum / sum-of-squares for one token chunk.

    lhsT is a [128,128] ones matrix, so every output partition carries the
    full sum. Returns PSUM tiles (sum_ps, ssq_ps) [128,TN] f32."""
    tsl = bass.ts(tcix, TN)
    sum_ps = psum.tile([128, TN], F32, tag=pstag, bufs=psbufs,
                       name=f"{tag}sum_ps")
    ssq_ps = psum.tile([128, TN], F32, tag=pstag, bufs=psbufs,
                       name=f"{tag}ssq_ps")
    for c in range(NC_CHUNKS):
        xr = pool.tile([128, TN], BF16, tag="xr", bufs=1, name=f"{tag}xr")
        nc.vector.tensor_copy(xr[:], x_tiles[c][:, tsl])
        sq = pool.tile([128, TN], BF16, tag="sq", bufs=1, name=f"{tag}sq")
        nc.vector.tensor_mul(sq[:], x_tiles[c][:, tsl], x_tiles[c][:, tsl])
        nc.tensor.matmul(sum_ps[:], ones_m[:], xr[:],
                         start=(c == 0), stop=(c == NC_CHUNKS - 1))
        nc.tensor.matmul(ssq_ps[:], ones_m[:], sq[:],
                         start=(c == 0), stop=(c == NC_CHUNKS - 1))
    return sum_ps, ssq_ps


def _ln_finish(nc, pool, eps_t, sums, tag):
    """mu/rstd on wide tiles straight from the broadcast-sum PSUM banks.

    Returns (mu_b, rstd_b) [128,TN] f32 SBUF tiles."""
    sum_ps, ssq_ps = sums
    mu_b = pool.tile([128, TN], F32, tag="stmu", bufs=1, name=f"{tag}mu")
    nc.vector.tensor_scalar_mul(mu_b[:], sum_ps[:], 1.0 / C)
    msq = pool.tile([128, TN], F32, tag="stmsq", bufs=1, name=f"{tag}msq")
    nc.vector.tensor_mul(msq[:], mu_b[:], mu_b[:])
    # var = ssq/C - mu^2 (into msq), then std, then rstd
    nc.vector.scalar_tensor_tensor(
        out=msq[:], in0=ssq_ps[:], scalar=1.0 / C, in1=msq[:],
        op0=mybir.AluOpType.mult, op1=mybir.AluOpType.subtract)
    nc.scalar.activation(out=msq[:], in_=msq[:],
                         func=mybir.ActivationFunctionType.Sqrt,
                         bias=eps_t[:, :])
    rstd_b = pool.tile([128, TN], F32, tag="strs", bufs=1, name=f"{tag}rstd")
    nc.vector.reciprocal_approx_fast(out=rstd_b[:], in_=msq[:])
    return mu_b, rstd_b


def _make_h(nc, pool, x_tiles, mu_b, rstd_b, tcix, tag):
    """h = (x - mu) * rstd in bf16 (in place, two DVE passes per chunk)."""
    tsl = bass.ts(tcix, TN)
    hp = []
    for c in range(NC_CHUNKS):
        h = pool.tile([128, TN], BF16, tag=tag, bufs=18, name=tag)
        nc.vector.tensor_sub(h[:], x_tiles[c][:, tsl], mu_b[:])
        nc.vector.tensor_mul(h[:], h[:], rstd_b[:])
        hp.append(h)
    return hp


def _emit(nc, tc_, x_in, wqk, wv, wproj, wfc, wfc2,
          bqk_d, pbias_d, bproj_d, bfc_d, bfc2_d, out_d):
    ts = bass.ts

    persist = tc_.alloc_tile_pool(name="persist", bufs=1)
    dram = tc_.alloc_tile_pool(name="dram", bufs=1, space="DRAM")

    # residual stream x: 6 fp32 tiles [128, T], DMA'd per token chunk so the
    # first LN doesn't wait on whole-tensor loads
    x_tiles = []
    for c in range(NC_CHUNKS):
        xt = persist.tile([128, T], F32, tag=f"x{c}", name=f"x{c}")
        x_tiles.append(xt)
    for tcix in range(TC):
        for c in range(NC_CHUNKS):
            nc.sync.dma_start(out=x_tiles[c][:, ts(tcix, TN)],
                              in_=x_in.ap()[ts(c, 128), ts(tcix, TN)])

    ones_m = persist.tile([128, 128], BF16, tag="ones_m", name="ones_m")
    nc.vector.memset(ones_m[:], 1.0)
    eps_t = persist.tile([128, 1], F32, tag="eps_t", name="eps_t")
    nc.vector.memset(eps_t[:], LN_EPS)

    def load_bias(dram_t, cols, nm):
        t = persist.tile([128, cols], F32, tag=nm, name=nm)
        nc.sync.dma_start(out=t[:], in_=dram_t.ap())
        return t

    bqk_sb = load_bias(bqk_d, 6, "bqk_sb")
    pbias_sb = load_bias(pbias_d, 6, "pbias_sb")
    bproj_sb = load_bias(bproj_d, 6, "bproj_sb")
    bfc_sb = load_bias(bfc_d, 12, "bfc_sb")
    bfc2_sb = load_bias(bfc2_d, 6, "bfc2_sb")

    # AllReduce bounce buffers (per token chunk)
    ar1_in = [dram.tile([C, TN], F32, tag=f"ar1i{t}", name=f"ar1i{t}")
              for t in range(TC)]
    ar1_out = [dram.tile([C, TN], F32, tag=f"ar1o{t}", name=f"ar1o{t}")
               for t in range(TC)]
    ar2_in = [dram.tile([C, TN], F32, tag=f"ar2i{t}", name=f"ar2i{t}")
              for t in range(TC)]
    ar2_out = [dram.tile([C, TN], F32, tag=f"ar2o{t}", name=f"ar2o{t}")
               for t in range(TC)]

    # weights whose pool outlives the attention pool: fc + proj
    mlpw = tc_.alloc_tile_pool(name="mlpw", bufs=1)
    wfc_sb = []
    for c in range(NC_CHUNKS):
        t = mlpw.tile([128, HID_LOC], BF16, tag=f"wfc{c}", name=f"wfc{c}")
        nc.sync.dma_start(out=t[:], in_=wfc.ap()[ts(c, 128), :])
        wfc_sb.append(t)
    wproj_sb = []
    for c in range(3):
        t = mlpw.tile([128, C], BF16, tag=f"wpj{c}", name=f"wpj{c}")
        nc.sync.dma_start(out=t[:], in_=wproj.ap()[ts(c, 128), :])
        wproj_sb.append(t)
    wfc2_sb = []
    for c in range(12):
        t = mlpw.tile([128, C], BF16, tag=f"wfc2_{c}", name=f"wfc2_{c}")
        nc.sync.dma_start(out=t[:], in_=wfc2.ap()[ts(c, 128), :])
        wfc2_sb.append(t)

    # ---------------- attention sublayer ----------------
    attn = tc_.alloc_tile_pool(name="attn", bufs=1)
    apsum = tc_.alloc_tile_pool(name="apsum", bufs=1, space="PSUM")

    wqk_sb = []
    for c in range(NC_CHUNKS):
        t = attn.tile([128, 2 * QKW], BF16, tag=f"wqk{c}", name=f"wqk{c}")
        nc.sync.dma_start(out=t[:], in_=wqk.ap()[ts(c, 128), :])
        wqk_sb.append(t)
    wv_sb = []
    for c in range(NC_CHUNKS):
        t = attn.tile([128, QKW], BF16, tag=f"wv{c}", name=f"wv{c}")
        nc.sync.dma_start(out=t[:], in_=wv.ap()[ts(c, 128), :])
        wv_sb.append(t)

    # q,k feature-major bf16 [128, T] x6 (first 3 = q chunks, last 3 = k chunks)
    qk_sb = [attn.tile([128, T], BF16, tag=f"qk{i}", name=f"qk{i}")
             for i in range(6)]
    # V token-major augmented with ones column: 16 tiles [128, 6*65] bf16
    vaug = [attn.tile([128, H_LOC * (HD + 1)], BF16, tag=f"va{i}", name=f"va{i}")
            for i in range(KT)]
    # normalized attention output, feature-major bf16 [384, T] as 3 tiles
    cvt_sb = [attn.tile([128, T], BF16, tag=f"cvt{i}", name=f"cvt{i}")
              for i in range(3)]

    # LN1 chains run two chunks ahead of the qkv GEMMs so the PE always has
    # stat sums or qkv matmuls ready while DVE normalize chains complete
    hps = {}

    def ln1_chain(tcix):
        sums = _ln_sums(nc, attn, apsum, x_tiles, ones_m, tcix, "l1",
                        "mmps", 3)
        mu_b, rstd_b = _ln_finish(nc, attn, eps_t, sums, "l1")
        hps[tcix] = _make_h(nc, attn, x_tiles, mu_b, rstd_b, tcix, "hp")

    ln1_chain(0)
    ln1_chain(1)

    for tcix in range(TC):
        tsl = ts(tcix, TN)
        if tcix + 2 < TC:
            ln1_chain(tcix + 2)
        hp = hps[tcix]
        # q,k feature-major
        for oc in range(6):
            ps = apsum.tile([128, TN], F32, tag="mmps", bufs=3, name="qkps")
            for c in range(NC_CHUNKS):
                nc.tensor.matmul(ps[:], wqk_sb[c][:, ts(oc, 128)], hp[c][:],
                                 start=(c == 0), stop=(c == NC_CHUNKS - 1))
            nc.vector.tensor_scalar_add(qk_sb[oc][:, tsl], ps[:],
                                        bqk_sb[:, oc:oc + 1])
        # V token-major (+ ones column)
        for s4 in range(TC):
            kt = tcix * 4 + s4
            vps = apsum.tile([128, QKW], F32, tag="mmps", bufs=3, name="vps")
            for c in range(NC_CHUNKS):
                nc.tensor.matmul(vps[:], hp[c][:, ts(s4, 128)], wv_sb[c][:],
                                 start=(c == 0), stop=(c == NC_CHUNKS - 1))
            va = vaug[kt]
            va_v = va[:].rearrange("p (h d) -> p h d", h=H_LOC)[:, :, 0:HD]
            nc.vector.tensor_copy(va_v, vps[:].rearrange("p (h d) -> p h d",
                                                         h=H_LOC))
            va_ones = va[:].rearrange("p (h d) -> p h d", h=H_LOC)[:, :, HD:HD + 1]
            nc.vector.memset(va_ones, 1.0)

    # attention proper + proj; big q chunks first so their AllReduce is in
    # flight the longest
    for qc in reversed(range(TC)):
        qsl = ts(qc, TN)
        for h in range(H_LOC):
            poff = (h % 2) * 64
            qh = qk_sb[h // 2][poff:poff + 64, qsl]
            khs = qk_sb[3 + h // 2]
            n_kc = 4 * (qc + 1)
            # S block: dense matmul stream; exp/mask chase on ACT/GpSimd
            pts = []
            for kc in range(n_kc):
                sps = apsum.tile([128, TN], F32, tag="sps", bufs=3, name="sps")
                nc.tensor.matmul(sps[:], khs[poff:poff + 64, ts(kc, 128)], qh,
                                 start=True, stop=True)
                pt = attn.tile([128, TN], BF16, tag="pt", bufs=8, name="pt")
                nc.scalar.activation(out=pt[:], in_=sps[:],
                                     func=mybir.ActivationFunctionType.Exp,
                                     scale=SCALE)
                j = kc - 4 * qc
                if j >= 0:
                    # causal band: zero cols < j*128, triangle in [j*128, +128)
                    w = j * 128 + 128
                    nc.gpsimd.affine_select(
                        out=pt[:, 0:w], in_=pt[:, 0:w],
                        pattern=[[1, w]],
                        compare_op=mybir.AluOpType.is_ge,
                        fill=0.0, base=-j * 128, channel_multiplier=-1)
                pts.append(pt)
            # PV block
            cvps = apsum.tile([HD + 1, TN], F32, tag="cvps", bufs=2, name="cvps")
            for kc in range(n_kc):
                nc.tensor.matmul(cvps[:], vaug[kc][:, ts(h, HD + 1)],
                                 pts[kc][:],
                                 start=(kc == 0), stop=(kc == n_kc - 1))
            # normalize by the ones-column sum
            rd = attn.tile([1, TN], F32, tag="rd", bufs=1, name="rd")
            nc.vector.tensor_copy(rd[:], cvps[HD:HD + 1, :])
            rd2 = attn.tile([1, TN], F32, tag="rd2", bufs=1, name="rd2")
            nc.vector.reciprocal_approx_fast(out=rd2[:], in_=rd[:])
            db = attn.tile([64, TN], F32, tag="db", bufs=1, name="db")
            nc.gpsimd.partition_broadcast(db[:], rd2[:])
            nc.vector.tensor_mul(cvt_sb[h // 2][poff:poff + 64, qsl],
                                 cvps[0:HD, :], db[:])
        # proj partials for this token chunk -> AR staging
        for oc in range(NC_CHUNKS):
            pps = apsum.tile([128, TN], F32, tag="mmps", bufs=3, name="pps")
            for c3 in range(3):
                nc.tensor.matmul(pps[:], wproj_sb[c3][:, ts(oc, 128)],
                                 cvt_sb[c3][:, qsl],
                                 start=(c3 == 0), stop=(c3 == 2))
            stg = attn.tile([128, TN], F32, tag="stg", bufs=1, name="stg")
            nc.vector.tensor_scalar_add(stg[:], pps[:], pbias_sb[:, oc:oc + 1])
            nc.sync.dma_start(out=ar1_in[qc][ts(oc, 128), :], in_=stg[:])
        nc.gpsimd.collective_compute(
            "AllReduce", mybir.AluOpType.add, replica_groups=REPLICA_GROUPS,
            ins=[ar1_in[qc].opt()], outs=[ar1_out[qc].opt()])

    attn.release()
    apsum.release()

    # ---------------- MLP sublayer ----------------
    mlp = tc_.alloc_tile_pool(name="mlp", bufs=1)
    mpsum = tc_.alloc_tile_pool(name="mpsum", bufs=1, space="PSUM")

    def residual1(tcix):
        tsl = ts(tcix, TN)
        for c in range(NC_CHUNKS):
            art = mlp.tile([128, TN], F32, tag="art", bufs=3, name="art")
            nc.sync.dma_start(out=art[:], in_=ar1_out[tcix][ts(c, 128), :])
            nc.vector.scalar_tensor_tensor(
                out=x_tiles[c][:, tsl], in0=art[:],
                scalar=bproj_sb[:, c:c + 1], in1=x_tiles[c][:, tsl],
                op0=mybir.AluOpType.add, op1=mybir.AluOpType.add)

    def make_h2(tcix):
        sums = _ln_sums(nc, mlp, mpsum, x_tiles, ones_m, tcix, "l2",
                        "fcps", 4)
        mu_b, rstd_b = _ln_finish(nc, mlp, eps_t, sums, "l2")
        return _make_h(nc, mlp, x_tiles, mu_b, rstd_b, tcix, "hp2")

    def fc_fc2_ar(tcix, hp):
        g_tiles = []
        for oc in range(12):
            ps = mpsum.tile([128, TN], F32, tag="fcps", bufs=4, name="fcps")
            for c in range(NC_CHUNKS):
                nc.tensor.matmul(ps[:], wfc_sb[c][:, ts(oc, 128)], hp[c][:],
                                 start=(c == 0), stop=(c == NC_CHUNKS - 1))
            g = mlp.tile([128, TN], BF16, tag="g", bufs=24, name="g")
            nc.scalar.activation(
                out=g[:], in_=ps[:],
                func=mybir.ActivationFunctionType.Gelu_apprx_tanh,
                bias=bfc_sb[:, oc:oc + 1])
            g_tiles.append(g)
        for oc in range(NC_CHUNKS):
            ps = mpsum.tile([128, TN], F32, tag="f2ps", bufs=4, name="f2ps")
            for c in range(12):
                nc.tensor.matmul(ps[:], wfc2_sb[c][:, ts(oc, 128)],
                                 g_tiles[c][:],
                                 start=(c == 0), stop=(c == 11))
            stg = mlp.tile([128, TN], F32, tag="stg2", bufs=2, name="stg2")
            nc.vector.tensor_copy(stg[:], ps[:])
            nc.sync.dma_start(out=ar2_in[tcix][ts(oc, 128), :], in_=stg[:])
        nc.gpsimd.collective_compute(
            "AllReduce", mybir.AluOpType.add, replica_groups=REPLICA_GROUPS,
            ins=[ar2_in[tcix].opt()], outs=[ar2_out[tcix].opt()])

    def residual2_store(tcix):
        tsl = ts(tcix, TN)
        for c in range(NC_CHUNKS):
            art = mlp.tile([128, TN], F32, tag="art2", bufs=3, name="art2")
            nc.sync.dma_start(out=art[:], in_=ar2_out[tcix][ts(c, 128), :])
            nc.vector.scalar_tensor_tensor(
                out=x_tiles[c][:, tsl], in0=art[:],
                scalar=bfc2_sb[:, c:c + 1], in1=x_tiles[c][:, tsl],
                op0=mybir.AluOpType.add, op1=mybir.AluOpType.add)
            nc.sync.dma_start(out=out_d.ap()[ts(c, 128), tsl],
                              in_=x_tiles[c][:, tsl])

    # tc3's AR1 fired first and is long done; tc0's AR1 is freshest, so its
    # residual is deferred until fc(3) keeps the PE busy
    h2 = {}
    for tcix in (3, 2, 1):
        residual1(tcix)
        h2[tcix] = make_h2(tcix)
    fc_fc2_ar(3, h2[3])
    residual1(0)
    h2[0] = make_h2(0)
    fc_fc2_ar(2, h2[2])
    fc_fc2_ar(1, h2[1])
    residual2_store(3)
    fc_fc2_ar(0, h2[0])
    residual2_store(2)
    residual2_store(1)
    residual2_store(0)

    mlp.release()
    mpsum.release()
    mlpw.release()
    persist.release()
    dram.release()


_NC_CACHE = None


def _get_nc():
    global _NC_CACHE
    if _NC_CACHE is None:
        _NC_CACHE = _build_nc()
    return _NC_CACHE


def _fold(v):
    return np.ascontiguousarray(v.reshape(-1, 128).T).astype(np.float32)


def _prep_core(core, x, ln1_g, ln1_b, w_attn, b_attn, w_proj, b_proj,
               ln2_g, ln2_b, w_fc, b_fc, w_fc2, b_fc2):
    b = core // 2
    tp = core % 2
    qs = slice(tp * QKW, (tp + 1) * QKW)
    ks = slice(C + tp * QKW, C + (tp + 1) * QKW)
    vs = slice(2 * C + tp * QKW, 2 * C + (tp + 1) * QKW)
    hs = slice(tp * HID_LOC, (tp + 1) * HID_LOC)

    x_fm = np.ascontiguousarray(x[b].T).astype(np.float32)

    wqk_h = np.concatenate([w_attn[:, qs], w_attn[:, ks]], axis=1)
    wqk_h = (wqk_h * ln1_g[:, None]).astype(np.float32)
    wv_h = (w_attn[:, vs] * ln1_g[:, None]).astype(np.float32)

    bqk = np.concatenate([b_attn[qs], b_attn[ks]]) + ln1_b @ np.concatenate(
        [w_attn[:, qs], w_attn[:, ks]], axis=1)
    bv = b_attn[vs] + ln1_b @ w_attn[:, vs]

    wproj_h = w_proj[tp * QKW:(tp + 1) * QKW, :]
    pbias = bv @ wproj_h                       # folded v-bias contribution
    wfc_h = (w_fc[:, hs] * ln2_g[:, None]).astype(np.float32)
    bfc = b_fc[hs] + ln2_b @ w_fc[:, hs]
    wfc2_h = w_fc2[hs, :]

    # b_proj / b_fc2 are added once per core after the AllReduce
    return {
        "x_fm": x_fm,
        "wqk": wqk_h.astype(ml_dtypes.bfloat16),
        "wv": wv_h.astype(ml_dtypes.bfloat16),
        "wproj": wproj_h.astype(ml_dtypes.bfloat16),
        "wfc": wfc_h.astype(ml_dtypes.bfloat16),
        "wfc2": wfc2_h.astype(ml_dtypes.bfloat16),
        "bqk": _fold(bqk),
        "pbias": _fold(pbias),
        "bproj": _fold(np.asarray(b_proj)),
        "bfc": _fold(np.asarray(b_fc)),
        "bfc2": _fold(np.asarray(b_fc2)),
    }


def kernel(x, ln1_g, ln1_b, w_attn, b_attn, w_proj, b_proj,
           ln2_g, ln2_b, w_fc, b_fc, w_fc2, b_fc2, _trace=False):
    args = [np.asarray(a, np.float32) for a in
            (x, ln1_g, ln1_b, w_attn, b_attn, w_proj, b_proj,
             ln2_g, ln2_b, w_fc, b_fc, w_fc2, b_fc2)]
    nc = _get_nc()
    in_maps = [_prep_core(core, *args) for core in range(N_CORES)]
    res = run_bass_kernel_spmd(nc, in_maps, list(range(N_CORES)),
                               trace=_trace)
    out = np.empty((B, T, C), np.float32)
    for b in range(B):
        out[b] = res.results[2 * b]["out_fm"].T
    kernel._last_result = res
    return out


# revision 28
# speedup vs baseline: 1.0828x; 1.0828x over previous
"""Trainium2 Bass kernel for a GPT-2-style transformer block.

Problem: x[4,2048,768] through pre-LN attention (12 heads, causal) + pre-LN MLP
(4x hidden, tanh-approx gelu), residual connections.

Sharding: 8 cores = 4 batch elements x 2-way tensor parallel (heads 0-5 / 6-11
for attention, hidden cols 0-1535 / 1536-3071 for the MLP). Pairwise AllReduce
{0,1}{2,3}{4,5}{6,7} after c_proj and after c_fc2, chunked by token blocks and
ordered so every collective lands before anything in an engine stream waits on
it (in-order engines suffer head-of-line blocking otherwise).

Device layout is feature-major ([C, T]: features on partitions, tokens on the
free dim). The host pre-transposes x, pre-folds LN gains/biases into the weight
matrices, and transposes the output back. LN partition-dim sums are matmuls
against a [128,128] ones matrix, which broadcasts the sums to every partition
for free; all stat math then runs on wide [128,TN] tiles (no single-partition
vector ops anywhere). Softmax uses exp without max subtraction (logits are O(1)
here) with denominators recovered through an appended ones-column on V, a fast
approximate reciprocal, and a partition broadcast applied to the small
attention output. Attention emits each head's S matmuls as one dense block and
the P@V accumulation as a second block so the exp/mask chain never bubbles the
PE. Matmuls run in bf16 (fp32 PSUM accumulation); the residual stream and all
normalization math stay fp32.
"""

import numpy as np
import ml_dtypes

import concourse.bacc as bacc
import concourse.bass as bass
import concourse.mybir as mybir
import concourse.tile as tile
from concourse.bass_utils import run_bass_kernel_spmd

N_CORES = 8
B, T, C = 4, 2048, 768
H = 12
HD = 64
HIDDEN = 4 * C
LN_EPS = 1e-5

NC_CHUNKS = C // 128          # 6 feature chunks
TC = 4                        # token chunks
TN = T // TC                  # 512 tokens per chunk
KT = T // 128                 # 16 k-subtiles
H_LOC = H // 2                # 6 heads per core
QKW = H_LOC * HD              # 384 per-core q/k/v width
HID_LOC = HIDDEN // 2         # 1536 per-core hidden
SCALE = 1.0 / 8.0             # 1/sqrt(64)

F32 = mybir.dt.float32
BF16 = mybir.dt.bfloat16

REPLICA_GROUPS = [[0, 1], [2, 3], [4, 5], [6, 7]]


def _build_nc():
    nc = bacc.Bacc("TRN2", target_bir_lowering=False, debug=False,
                   num_devices=N_CORES)

    x_in = nc.dram_tensor("x_fm", [C, T], F32, kind="ExternalInput")
    wqk = nc.dram_tensor("wqk", [C, 2 * QKW], BF16, kind="ExternalInput")
    wv = nc.dram_tensor("wv", [C, QKW], BF16, kind="ExternalInput")
    wproj = nc.dram_tensor("wproj", [QKW, C], BF16, kind="ExternalInput")
    wfc = nc.dram_tensor("wfc", [C, HID_LOC], BF16, kind="ExternalInput")
    wfc2 = nc.dram_tensor("wfc2", [HID_LOC, C], BF16, kind="ExternalInput")
    bqk_d = nc.dram_tensor("bqk", [128, 6], F32, kind="ExternalInput")
    pbias_d = nc.dram_tensor("pbias", [128, 6], F32, kind="ExternalInput")
    bproj_d = nc.dram_tensor("bproj", [128, 6], F32, kind="ExternalInput")
    bfc_d = nc.dram_tensor("bfc", [128, 12], F32, kind="ExternalInput")
    bfc2_d = nc.dram_tensor("bfc2", [128, 6], F32, kind="ExternalInput")
    out_d = nc.dram_tensor("out_fm", [C, T], F32, kind="ExternalOutput")

    with tile.TileContext(nc) as tc_:
        _emit(nc, tc_, x_in, wqk, wv, wproj, wfc, wfc2,
              bqk_d, pbias_d, bproj_d, bfc_d, bfc2_d, out_d)

    nc.compile()
    return nc


def _ln_sums(nc, pool, psum, x_tiles, ones_m, tcix, tag, pstag, psbufs):
    """Broadcast partition-dim sum / sum-of-squares for one token chunk.

    lhsT is a [128,128] ones matrix, so every output partition carries the
    full sum. Returns PSUM tiles (sum_ps, ssq_ps) [128,TN] f32."""
    tsl = bass.ts(tcix, TN)
    sum_ps = psum.tile([128, TN], F32, tag=pstag, bufs=psbufs,
                       name=f"{tag}sum_ps")
    ssq_ps = psum.tile([128, TN], F32, tag=pstag, bufs=psbufs,
                       name=f"{tag}ssq_ps")
    for c in range(NC_CHUNKS):
        xr = pool.tile([128, TN], BF16, tag="xr", bufs=2, name=f"{tag}xr")
        nc.vector.tensor_copy(xr[:], x_tiles[c][:, tsl])
        sq = pool.tile([128, TN], BF16, tag="sq", bufs=2, name=f"{tag}sq")
        nc.vector.tensor_mul(sq[:], x_tiles[c][:, tsl], x_tiles[c][:, tsl])
        nc.tensor.matmul(sum_ps[:], ones_m[:], xr[:],
                         start=(c == 0), stop=(c == NC_CHUNKS - 1))
        nc.tensor.matmul(ssq_ps[:], ones_m[:], sq[:],
                         start=(c == 0), stop=(c == NC_CHUNKS - 1))
    return sum_ps, ssq_ps


def _ln_finish(nc, pool, eps_t, sums, tag):
    """mu/rstd on wide tiles straight from the broadcast-sum PSUM banks.

    Returns (mu_b, rstd_b) [128,TN] f32 SBUF tiles."""
    sum_ps, ssq_ps = sums
    mu_b = pool.tile([128, TN], F32, tag="stmu", bufs=1, name=f"{tag}mu")
    nc.vector.tensor_scalar_mul(mu_b[:], sum_ps[:], 1.0 / C)
    msq = pool.tile([128, TN], F32, tag="stmsq", bufs=1, name=f"{tag}msq")
    nc.vector.tensor_mul(msq[:], mu_b[:], mu_b[:])
    # var = ssq/C - mu^2 (into msq), then std, then rstd
    nc.vector.scalar_tensor_tensor(
        out=msq[:], in0=ssq_ps[:], scalar=1.0 / C, in1=msq[:],
        op0=mybir.AluOpType.mult, op1=mybir.AluOpType.subtract)
    nc.scalar.activation(out=msq[:], in_=msq[:],
                         func=mybir.ActivationFunctionType.Sqrt,
                         bias=eps_t[:, :])
    rstd_b = pool.tile([128, TN], F32, tag="strs", bufs=1, name=f"{tag}rstd")
    nc.vector.reciprocal_approx_fast(out=rstd_b[:], in_=msq[:])
    return mu_b, rstd_b


def _make_h(nc, pool, x_tiles, mu_b, rstd_b, tcix, tag):
    """h = (x - mu) * rstd in bf16 (in place, two DVE passes per chunk)."""
    tsl = bass.ts(tcix, TN)
    hp = []
    for c in range(NC_CHUNKS):
        h = pool.tile([128, TN], BF16, tag=tag, bufs=18, name=tag)
        nc.vector.tensor_sub(h[:], x_tiles[c][:, tsl], mu_b[:])
        nc.vector.tensor_mul(h[:], h[:], rstd_b[:])
        hp.append(h)
    return hp


def _emit(nc, tc_, x_in, wqk, wv, wproj, wfc, wfc2,
          bqk_d, pbias_d, bproj_d, bfc_d, bfc2_d, out_d):
    ts = bass.ts

    persist = tc_.alloc_tile_pool(name="persist", bufs=1)
    dram = tc_.alloc_tile_pool(name="dram", bufs=1, space="DRAM")

    # residual stream x: 6 fp32 tiles [128, T], DMA'd per token chunk so the
    # first LN doesn't wait on whole-tensor loads
    x_tiles = []
    for c in range(NC_CHUNKS):
        xt = persist.tile([128, T], F32, tag=f"x{c}", name=f"x{c}")
        x_tiles.append(xt)
    for tcix in range(TC):
        for c in range(NC_CHUNKS):
            nc.sync.dma_start(out=x_tiles[c][:, ts(tcix, TN)],
                              in_=x_in.ap()[ts(c, 128), ts(tcix, TN)])

    ones_m = persist.tile([128, 128], BF16, tag="ones_m", name="ones_m")
    nc.vector.memset(ones_m[:], 1.0)
    eps_t = persist.tile([128, 1], F32, tag="eps_t", name="eps_t")
    nc.vector.memset(eps_t[:], LN_EPS)

    def load_bias(dram_t, cols, nm):
        t = persist.tile([128, cols], F32, tag=nm, name=nm)
        nc.sync.dma_start(out=t[:], in_=dram_t.ap())
        return t

    bqk_sb = load_bias(bqk_d, 6, "bqk_sb")
    pbias_sb = load_bias(pbias_d, 6, "pbias_sb")
    bproj_sb = load_bias(bproj_d, 6, "bproj_sb")
    bfc_sb = load_bias(bfc_d, 12, "bfc_sb")
    bfc2_sb = load_bias(bfc2_d, 6, "bfc2_sb")

    # AllReduce bounce buffers (per token chunk)
    ar1_in = [dram.tile([C, TN], F32, tag=f"ar1i{t}", name=f"ar1i{t}")
              for t in range(TC)]
    ar1_out = [dram.tile([C, TN], F32, tag=f"ar1o{t}", name=f"ar1o{t}")
               for t in range(TC)]
    ar2_in = [dram.tile([C, TN], F32, tag=f"ar2i{t}", name=f"ar2i{t}")
              for t in range(TC)]
    ar2_out = [dram.tile([C, TN], F32, tag=f"ar2o{t}", name=f"ar2o{t}")
               for t in range(TC)]

    # weights whose pool outlives the attention pool: fc + proj
    mlpw = tc_.alloc_tile_pool(name="mlpw", bufs=1)
    wfc_sb = []
    for c in range(NC_CHUNKS):
        t = mlpw.tile([128, HID_LOC], BF16, tag=f"wfc{c}", name=f"wfc{c}")
        nc.sync.dma_start(out=t[:], in_=wfc.ap()[ts(c, 128), :])
        wfc_sb.append(t)
    wproj_sb = []
    for c in range(3):
        t = mlpw.tile([128, C], BF16, tag=f"wpj{c}", name=f"wpj{c}")
        nc.sync.dma_start(out=t[:], in_=wproj.ap()[ts(c, 128), :])
        wproj_sb.append(t)
    wfc2_sb = []
    for c in range(12):
        t = mlpw.tile([128, C], BF16, tag=f"wfc2_{c}", name=f"wfc2_{c}")
        nc.sync.dma_start(out=t[:], in_=wfc2.ap()[ts(c, 128), :])
        wfc2_sb.append(t)

    # ---------------- attention sublayer ----------------
    attn = tc_.alloc_tile_pool(name="attn", bufs=1)
    apsum = tc_.alloc_tile_pool(name="apsum", bufs=1, space="PSUM")

    wqk_sb = []
    for c in range(NC_CHUNKS):
        t = attn.tile([128, 2 * QKW], BF16, tag=f"wqk{c}", name=f"wqk{c}")
        nc.sync.dma_start(out=t[:], in_=wqk.ap()[ts(c, 128), :])
        wqk_sb.append(t)
    wv_sb = []
    for c in range(NC_CHUNKS):
        t = attn.tile([128, QKW], BF16, tag=f"wv{c}", name=f"wv{c}")
        nc.sync.dma_start(out=t[:], in_=wv.ap()[ts(c, 128), :])
        wv_sb.append(t)

    # q,k feature-major bf16 [128, T] x6 (first 3 = q chunks, last 3 = k chunks)
    qk_sb = [attn.tile([128, T], BF16, tag=f"qk{i}", name=f"qk{i}")
             for i in range(6)]
    # V token-major augmented with ones column: 16 tiles [128, 6*65] bf16
    vaug = [attn.tile([128, H_LOC * (HD + 1)], BF16, tag=f"va{i}", name=f"va{i}")
            for i in range(KT)]
    # normalized attention output, feature-major bf16 [384, T] as 3 tiles
    cvt_sb = [attn.tile([128, T], BF16, tag=f"cvt{i}", name=f"cvt{i}")
              for i in range(3)]

    # LN1 chains run two chunks ahead of the qkv GEMMs so the PE always has
    # stat sums or qkv matmuls ready while DVE normalize chains complete
    hps = {}

    def ln1_chain(tcix):
        sums = _ln_sums(nc, attn, apsum, x_tiles, ones_m, tcix, "l1",
                        "mmps", 3)
        mu_b, rstd_b = _ln_finish(nc, attn, eps_t, sums, "l1")
        hps[tcix] = _make_h(nc, attn, x_tiles, mu_b, rstd_b, tcix, "hp")

    ln1_chain(0)
    ln1_chain(1)

    for tcix in range(TC):
        tsl = ts(tcix, TN)
        if tcix + 2 < TC:
            ln1_chain(tcix + 2)
        hp = hps[tcix]
        # q,k feature-major
        for oc in range(6):
            ps = apsum.tile([128, TN], F32, tag="mmps", bufs=3, name="qkps")
            for c in range(NC_CHUNKS):
                nc.tensor.matmul(ps[:], wqk_sb[c][:, ts(oc, 128)], hp[c][:],
                                 start=(c == 0), stop=(c == NC_CHUNKS - 1))
            nc.vector.tensor_scalar_add(qk_sb[oc][:, tsl], ps[:],
                                        bqk_sb[:, oc:oc + 1])
        # V token-major (+ ones column)
        for s4 in range(TC):
            kt = tcix * 4 + s4
            vps = apsum.tile([128, QKW], F32, tag="mmps", bufs=3, name="vps")
            for c in range(NC_CHUNKS):
                nc.tensor.matmul(vps[:], hp[c][:, ts(s4, 128)], wv_sb[c][:],
                                 start=(c == 0), stop=(c == NC_CHUNKS - 1))
            va = vaug[kt]
            va_v = va[:].rearrange("p (h d) -> p h d", h=H_LOC)[:, :, 0:HD]
            nc.vector.tensor_copy(va_v, vps[:].rearrange("p (h d) -> p h d",
                                                         h=H_LOC))
            va_ones = va[:].rearrange("p (h d) -> p h d", h=H_LOC)[:, :, HD:HD + 1]
            nc.vector.memset(va_ones, 1.0)

    # attention proper + proj; big q chunks first so their AllReduce is in
    # flight the longest
    for qc in reversed(range(TC)):
        qsl = ts(qc, TN)
        for h in range(H_LOC):
            poff = (h % 2) * 64
            qh = qk_sb[h // 2][poff:poff + 64, qsl]
            khs = qk_sb[3 + h // 2]
            n_kc = 4 * (qc + 1)
            # S block: dense matmul stream; exp/mask chase on ACT/GpSimd
            pts = []
            for kc in range(n_kc):
                sps = apsum.tile([128, TN], F32, tag="sps", bufs=3, name="sps")
                nc.tensor.matmul(sps[:], khs[poff:poff + 64, ts(kc, 128)], qh,
                                 start=True, stop=True)
                pt = attn.tile([128, TN], BF16, tag="pt", bufs=14, name="pt")
                nc.scalar.activation(out=pt[:], in_=sps[:],
                                     func=mybir.ActivationFunctionType.Exp,
                                     scale=SCALE)
                j = kc - 4 * qc
                if j >= 0:
                    # causal band: zero cols < j*128, triangle in [j*128, +128)
                    w = j * 128 + 128
                    nc.gpsimd.affine_select(
                        out=pt[:, 0:w], in_=pt[:, 0:w],
                        pattern=[[1, w]],
                        compare_op=mybir.AluOpType.is_ge,
                        fill=0.0, base=-j * 128, channel_multiplier=-1)
                pts.append(pt)
            # PV block
            cvps = apsum.tile([HD + 1, TN], F32, tag="cvps", bufs=2, name="cvps")
            for kc in range(n_kc):
                nc.tensor.matmul(cvps[:], vaug[kc][:, ts(h, HD + 1)],
                                 pts[kc][:],
                                 start=(kc == 0), stop=(kc == n_kc - 1))
            # normalize by the ones-column sum
            rd = attn.tile([1, TN], F32, tag="rd", bufs=1, name="rd")
            nc.vector.tensor_copy(rd[:], cvps[HD:HD + 1, :])
            rd2 = attn.tile([1, TN], F32, tag="rd2", bufs=1, name="rd2")
            nc.vector.reciprocal_approx_fast(out=rd2[:], in_=rd[:])
            db = attn.tile([64, TN], F32, tag="db", bufs=1, name="db")
            nc.gpsimd.partition_broadcast(db[:], rd2[:])
            nc.vector.tensor_mul(cvt_sb[h // 2][poff:poff + 64, qsl],
                                 cvps[0:HD, :], db[:])
        # proj partials for this token chunk -> AR staging
        for oc in range(NC_CHUNKS):
            pps = apsum.tile([128, TN], F32, tag="mmps", bufs=3, name="pps")
            for c3 in range(3):
                nc.tensor.matmul(pps[:], wproj_sb[c3][:, ts(oc, 128)],
                                 cvt_sb[c3][:, qsl],
                                 start=(c3 == 0), stop=(c3 == 2))
            stg = attn.tile([128, TN], F32, tag="stg", bufs=2, name="stg")
            nc.vector.tensor_scalar_add(stg[:], pps[:], pbias_sb[:, oc:oc + 1])
            nc.sync.dma_start(out=ar1_in[qc][ts(oc, 128), :], in_=stg[:])
        nc.gpsimd.collective_compute(
            "AllReduce", mybir.AluOpType.add, replica_groups=REPLICA_GROUPS,
            ins=[ar1_in[qc].opt()], outs=[ar1_out[qc].opt()])

    attn.release()
    apsum.release()

    # ---------------- MLP sublayer ----------------
    mlp = tc_.alloc_tile_pool(name="mlp", bufs=1)
    mpsum = tc_.alloc_tile_pool(name="mpsum", bufs=1, space="PSUM")

    def residual1(tcix):
        tsl = ts(tcix, TN)
        for c in range(NC_CHUNKS):
            art = mlp.tile([128, TN], F32, tag="art", bufs=3, name="art")
            nc.sync.dma_start(out=art[:], in_=ar1_out[tcix][ts(c, 128), :])
            nc.vector.scalar_tensor_tensor(
                out=x_tiles[c][:, tsl], in0=art[:],
                scalar=bproj_sb[:, c:c + 1], in1=x_tiles[c][:, tsl],
                op0=mybir.AluOpType.add, op1=mybir.AluOpType.add)

    def make_h2(tcix):
        sums = _ln_sums(nc, mlp, mpsum, x_tiles, ones_m, tcix, "l2",
                        "fcps", 4)
        mu_b, rstd_b = _ln_finish(nc, mlp, eps_t, sums, "l2")
        return _make_h(nc, mlp, x_tiles, mu_b, rstd_b, tcix, "hp2")

    def fc_fc2_ar(tcix, hp):
        g_tiles = []
        for oc in range(12):
            ps = mpsum.tile([128, TN], F32, tag="fcps", bufs=4, name="fcps")
            for c in range(NC_CHUNKS):
                nc.tensor.matmul(ps[:], wfc_sb[c][:, ts(oc, 128)], hp[c][:],
                                 start=(c == 0), stop=(c == NC_CHUNKS - 1))
            g = mlp.tile([128, TN], BF16, tag="g", bufs=24, name="g")
            nc.scalar.activation(
                out=g[:], in_=ps[:],
                func=mybir.ActivationFunctionType.Gelu_apprx_tanh,
                bias=bfc_sb[:, oc:oc + 1])
            g_tiles.append(g)
        for oc in range(NC_CHUNKS):
            ps = mpsum.tile([128, TN], F32, tag="f2ps", bufs=4, name="f2ps")
            for c in range(12):
                nc.tensor.matmul(ps[:], wfc2_sb[c][:, ts(oc, 128)],
                                 g_tiles[c][:],
                                 start=(c == 0), stop=(c == 11))
            stg = mlp.tile([128, TN], F32, tag="stg2", bufs=2, name="stg2")
            nc.vector.tensor_copy(stg[:], ps[:])
            nc.sync.dma_start(out=ar2_in[tcix][ts(oc, 128), :], in_=stg[:])
        nc.gpsimd.collective_compute(
            "AllReduce", mybir.AluOpType.add, replica_groups=REPLICA_GROUPS,
            ins=[ar2_in[tcix].opt()], outs=[ar2_out[tcix].opt()])

    def residual2_store(tcix):
        tsl = ts(tcix, TN)
        for c in range(NC_CHUNKS):
            art = mlp.tile([128, TN], F32, tag="art2", bufs=3, name="art2")
            nc.sync.dma_start(out=art[:], in_=ar2_out[tcix][ts(c, 128), :])
            nc.vector.scalar_tensor_tensor(
                out=x_tiles[c][:, tsl], in0=art[:],
                scalar=bfc2_sb[:, c:c + 1], in1=x_tiles[c][:, tsl],
                op0=mybir.AluOpType.add, op1=mybir.AluOpType.add)
            nc.sync.dma_start(out=out_d.ap()[ts(c, 128), tsl],
                              in_=x_tiles[c][:, tsl])

    # tc3's AR1 fired first and is long done; tc0's AR1 is freshest, so its
    # residual is deferred until fc(3) keeps the PE busy
    h2 = {}
    for tcix in (3, 2, 1):
        residual1(tcix)
        h2[tcix] = make_h2(tcix)
    fc_fc2_ar(3, h2[3])
    residual1(0)
    h2[0] = make_h2(0)
    fc_fc2_ar(2, h2[2])
    fc_fc2_ar(1, h2[1])
    residual2_store(3)
    fc_fc2_ar(0, h2[0])
    residual2_store(2)
    residual2_store(1)
    residual2_store(0)

    mlp.release()
    mpsum.release()
    mlpw.release()
    persist.release()
    dram.release()


_NC_CACHE = None


def _get_nc():
    global _NC_CACHE
    if _NC_CACHE is None:
        _NC_CACHE = _build_nc()
    return _NC_CACHE


def _fold(v):
    return np.ascontiguousarray(v.reshape(-1, 128).T).astype(np.float32)


def _prep_core(core, x, ln1_g, ln1_b, w_attn, b_attn, w_proj, b_proj,
               ln2_g, ln2_b, w_fc, b_fc, w_fc2, b_fc2):
    b = core // 2
    tp = core % 2
    qs = slice(tp * QKW, (tp + 1) * QKW)
    ks = slice(C + tp * QKW, C + (tp + 1) * QKW)
    vs = slice(2 * C + tp * QKW, 2 * C + (tp + 1) * QKW)
    hs = slice(tp * HID_LOC, (tp + 1) * HID_LOC)

    x_fm = np.ascontiguousarray(x[b].T).astype(np.float32)

    wqk_h = np.concatenate([w_attn[:, qs], w_attn[:, ks]], axis=1)
    wqk_h = (wqk_h * ln1_g[:, None]).astype(np.float32)
    wv_h = (w_attn[:, vs] * ln1_g[:, None]).astype(np.float32)

    bqk = np.concatenate([b_attn[qs], b_attn[ks]]) + ln1_b @ np.concatenate(
        [w_attn[:, qs], w_attn[:, ks]], axis=1)
    bv = b_attn[vs] + ln1_b @ w_attn[:, vs]

    wproj_h = w_proj[tp * QKW:(tp + 1) * QKW, :]
    pbias = bv @ wproj_h                       # folded v-bias contribution
    wfc_h = (w_fc[:, hs] * ln2_g[:, None]).astype(np.float32)
    bfc = b_fc[hs] + ln2_b @ w_fc[:, hs]
    wfc2_h = w_fc2[hs, :]

    # b_proj / b_fc2 are added once per core after the AllReduce
    return {
        "x_fm": x_fm,
        "wqk": wqk_h.astype(ml_dtypes.bfloat16),
        "wv": wv_h.astype(ml_dtypes.bfloat16),
        "wproj": wproj_h.astype(ml_dtypes.bfloat16),
        "wfc": wfc_h.astype(ml_dtypes.bfloat16),
        "wfc2": wfc2_h.astype(ml_dtypes.bfloat16),
        "bqk": _fold(bqk),
        "pbias": _fold(pbias),
        "bproj": _fold(np.asarray(b_proj)),
        "bfc": _fold(np.asarray(b_fc)),
        "bfc2": _fold(np.asarray(b_fc2)),
    }


def kernel(x, ln1_g, ln1_b, w_attn, b_attn, w_proj, b_proj,
           ln2_g, ln2_b, w_fc, b_fc, w_fc2, b_fc2, _trace=False):
    args = [np.asarray(a, np.float32) for a in
            (x, ln1_g, ln1_b, w_attn, b_attn, w_proj, b_proj,
             ln2_g, ln2_b, w_fc, b_fc, w_fc2, b_fc2)]
    nc = _get_nc()
    in_maps = [_prep_core(core, *args) for core in range(N_CORES)]
    res = run_bass_kernel_spmd(nc, in_maps, list(range(N_CORES)),
                               trace=_trace)
    out = np.empty((B, T, C), np.float32)
    for b in range(B):
        out[b] = res.results[2 * b]["out_fm"].T
    kernel._last_result = res
    return out


# revision 29
# speedup vs baseline: 1.0874x; 1.0042x over previous
"""Trainium2 Bass kernel for a GPT-2-style transformer block.

Problem: x[4,2048,768] through pre-LN attention (12 heads, causal) + pre-LN MLP
(4x hidden, tanh-approx gelu), residual connections.

Sharding: 8 cores = 4 batch elements x 2-way tensor parallel (heads 0-5 / 6-11
for attention, hidden cols 0-1535 / 1536-3071 for the MLP). Pairwise AllReduce
{0,1}{2,3}{4,5}{6,7} after c_proj and after c_fc2, chunked by token blocks and
ordered so every collective lands before anything in an engine stream waits on
it (in-order engines suffer head-of-line blocking otherwise).

Device layout is feature-major ([C, T]: features on partitions, tokens on the
free dim). The host pre-transposes x, pre-folds LN gains/biases into the weight
matrices, and transposes the output back. LN partition-dim sums are matmuls
against a [128,128] ones matrix, which broadcasts the sums to every partition
for free; all stat math then runs on wide [128,TN] tiles (no single-partition
vector ops anywhere). Softmax uses exp without max subtraction (logits are O(1)
here) with denominators recovered through an appended ones-column on V, a fast
approximate reciprocal, and a partition broadcast applied to the small
attention output. Attention emits each head's S matmuls as one dense block and
the P@V accumulation as a second block so the exp/mask chain never bubbles the
PE. Matmuls run in bf16 (fp32 PSUM accumulation); the residual stream and all
normalization math stay fp32.
"""

import numpy as np
import ml_dtypes

import concourse.bacc as bacc
import concourse.bass as bass
import concourse.mybir as mybir
import concourse.tile as tile
from concourse.bass_utils import run_bass_kernel_spmd

N_CORES = 8
B, T, C = 4, 2048, 768
H = 12
HD = 64
HIDDEN = 4 * C
LN_EPS = 1e-5

NC_CHUNKS = C // 128          # 6 feature chunks
TC = 4                        # token chunks
TN = T // TC                  # 512 tokens per chunk
KT = T // 128                 # 16 k-subtiles
H_LOC = H // 2                # 6 heads per core
QKW = H_LOC * HD              # 384 per-core q/k/v width
HID_LOC = HIDDEN // 2         # 1536 per-core hidden
SCALE = 1.0 / 8.0             # 1/sqrt(64)

F32 = mybir.dt.float32
BF16 = mybir.dt.bfloat16

REPLICA_GROUPS = [[0, 1], [2, 3], [4, 5], [6, 7]]


def _build_nc():
    nc = bacc.Bacc("TRN2", target_bir_lowering=False, debug=False,
                   num_devices=N_CORES)

    x_in = nc.dram_tensor("x_fm", [C, T], F32, kind="ExternalInput")
    wqk = nc.dram_tensor("wqk", [C, 2 * QKW], BF16, kind="ExternalInput")
    wv = nc.dram_tensor("wv", [C, QKW], BF16, kind="ExternalInput")
    wproj = nc.dram_tensor("wproj", [QKW, C], BF16, kind="ExternalInput")
    wfc = nc.dram_tensor("wfc", [C, HID_LOC], BF16, kind="ExternalInput")
    wfc2 = nc.dram_tensor("wfc2", [HID_LOC, C], BF16, kind="ExternalInput")
    bqk_d = nc.dram_tensor("bqk", [128, 6], F32, kind="ExternalInput")
    pbias_d = nc.dram_tensor("pbias", [128, 6], F32, kind="ExternalInput")
    bproj_d = nc.dram_tensor("bproj", [128, 6], F32, kind="ExternalInput")
    bfc_d = nc.dram_tensor("bfc", [128, 12], F32, kind="ExternalInput")
    bfc2_d = nc.dram_tensor("bfc2", [128, 6], F32, kind="ExternalInput")
    out_d = nc.dram_tensor("out_fm", [C, T], F32, kind="ExternalOutput")

    with tile.TileContext(nc) as tc_:
        _emit(nc, tc_, x_in, wqk, wv, wproj, wfc, wfc2,
              bqk_d, pbias_d, bproj_d, bfc_d, bfc2_d, out_d)

    nc.compile()
    return nc


def _ln_sums(nc, pool, psum, x_tiles, ones_m, tcix, tag, pstag, psbufs):
    """Broadcast partition-dim sum / sum-of-squares for one token chunk.

    lhsT is a [128,128] ones matrix, so every output partition carries the
    full sum. Returns PSUM tiles (sum_ps, ssq_ps) [128,TN] f32."""
    tsl = bass.ts(tcix, TN)
    sum_ps = psum.tile([128, TN], F32, tag=pstag, bufs=psbufs,
                       name=f"{tag}sum_ps")
    ssq_ps = psum.tile([128, TN], F32, tag=pstag, bufs=psbufs,
                       name=f"{tag}ssq_ps")
    for c in range(NC_CHUNKS):
        xr = pool.tile([128, TN], BF16, tag="xr", bufs=2, name=f"{tag}xr")
        nc.vector.tensor_copy(xr[:], x_tiles[c][:, tsl])
        sq = pool.tile([128, TN], BF16, tag="sq", bufs=2, name=f"{tag}sq")
        nc.vector.tensor_mul(sq[:], x_tiles[c][:, tsl], x_tiles[c][:, tsl])
        nc.tensor.matmul(sum_ps[:], ones_m[:], xr[:],
                         start=(c == 0), stop=(c == NC_CHUNKS - 1))
        nc.tensor.matmul(ssq_ps[:], ones_m[:], sq[:],
                         start=(c == 0), stop=(c == NC_CHUNKS - 1))
    return sum_ps, ssq_ps


def _ln_finish(nc, pool, eps_t, sums, tag):
    """mu/rstd on wide tiles straight from the broadcast-sum PSUM banks.

    Returns (mu_b, rstd_b) [128,TN] f32 SBUF tiles."""
    sum_ps, ssq_ps = sums
    mu_b = pool.tile([128, TN], F32, tag="stmu", bufs=1, name=f"{tag}mu")
    nc.vector.tensor_scalar_mul(mu_b[:], sum_ps[:], 1.0 / C)
    msq = pool.tile([128, TN], F32, tag="stmsq", bufs=1, name=f"{tag}msq")
    nc.vector.tensor_mul(msq[:], mu_b[:], mu_b[:])
    # var = ssq/C - mu^2 (into msq), then std, then rstd
    nc.vector.scalar_tensor_tensor(
        out=msq[:], in0=ssq_ps[:], scalar=1.0 / C, in1=msq[:],
        op0=mybir.AluOpType.mult, op1=mybir.AluOpType.subtract)
    nc.scalar.activation(out=msq[:], in_=msq[:],
                         func=mybir.ActivationFunctionType.Sqrt,
                         bias=eps_t[:, :])
    rstd_b = pool.tile([128, TN], F32, tag="strs", bufs=1, name=f"{tag}rstd")
    nc.vector.reciprocal_approx_fast(out=rstd_b[:], in_=msq[:])
    return mu_b, rstd_b


def _make_h(nc, pool, x_tiles, mu_b, rstd_b, tcix, tag):
    """h = (x - mu) * rstd in bf16 (in place, two DVE passes per chunk)."""
    tsl = bass.ts(tcix, TN)
    hp = []
    for c in range(NC_CHUNKS):
        h = pool.tile([128, TN], BF16, tag=tag, bufs=18, name=tag)
        nc.vector.tensor_sub(h[:], x_tiles[c][:, tsl], mu_b[:])
        nc.vector.tensor_mul(h[:], h[:], rstd_b[:])
        hp.append(h)
    return hp


def _emit(nc, tc_, x_in, wqk, wv, wproj, wfc, wfc2,
          bqk_d, pbias_d, bproj_d, bfc_d, bfc2_d, out_d):
    ts = bass.ts

    persist = tc_.alloc_tile_pool(name="persist", bufs=1)
    dram = tc_.alloc_tile_pool(name="dram", bufs=1, space="DRAM")

    # residual stream x: 6 fp32 tiles [128, T], DMA'd per token chunk so the
    # first LN doesn't wait on whole-tensor loads
    x_tiles = []
    for c in range(NC_CHUNKS):
        xt = persist.tile([128, T], F32, tag=f"x{c}", name=f"x{c}")
        x_tiles.append(xt)
    for tcix in range(TC):
        for c in range(NC_CHUNKS):
            nc.sync.dma_start(out=x_tiles[c][:, ts(tcix, TN)],
                              in_=x_in.ap()[ts(c, 128), ts(tcix, TN)])

    ones_m = persist.tile([128, 128], BF16, tag="ones_m", name="ones_m")
    nc.vector.memset(ones_m[:], 1.0)
    eps_t = persist.tile([128, 1], F32, tag="eps_t", name="eps_t")
    nc.vector.memset(eps_t[:], LN_EPS)

    def load_bias(dram_t, cols, nm):
        t = persist.tile([128, cols], F32, tag=nm, name=nm)
        nc.sync.dma_start(out=t[:], in_=dram_t.ap())
        return t

    bqk_sb = load_bias(bqk_d, 6, "bqk_sb")
    pbias_sb = load_bias(pbias_d, 6, "pbias_sb")
    bproj_sb = load_bias(bproj_d, 6, "bproj_sb")
    bfc_sb = load_bias(bfc_d, 12, "bfc_sb")
    bfc2_sb = load_bias(bfc2_d, 6, "bfc2_sb")

    # AllReduce bounce buffers (per token chunk)
    ar1_in = [dram.tile([C, TN], F32, tag=f"ar1i{t}", name=f"ar1i{t}")
              for t in range(TC)]
    ar1_out = [dram.tile([C, TN], F32, tag=f"ar1o{t}", name=f"ar1o{t}")
               for t in range(TC)]
    ar2_in = [dram.tile([C, TN], F32, tag=f"ar2i{t}", name=f"ar2i{t}")
              for t in range(TC)]
    ar2_out = [dram.tile([C, TN], F32, tag=f"ar2o{t}", name=f"ar2o{t}")
               for t in range(TC)]

    # weights whose pool outlives the attention pool: fc + proj
    mlpw = tc_.alloc_tile_pool(name="mlpw", bufs=1)
    wfc_sb = []
    for c in range(NC_CHUNKS):
        t = mlpw.tile([128, HID_LOC], BF16, tag=f"wfc{c}", name=f"wfc{c}")
        nc.sync.dma_start(out=t[:], in_=wfc.ap()[ts(c, 128), :])
        wfc_sb.append(t)
    wproj_sb = []
    for c in range(3):
        t = mlpw.tile([128, C], BF16, tag=f"wpj{c}", name=f"wpj{c}")
        nc.sync.dma_start(out=t[:], in_=wproj.ap()[ts(c, 128), :])
        wproj_sb.append(t)
    wfc2_sb = []
    for c in range(12):
        t = mlpw.tile([128, C], BF16, tag=f"wfc2_{c}", name=f"wfc2_{c}")
        nc.sync.dma_start(out=t[:], in_=wfc2.ap()[ts(c, 128), :])
        wfc2_sb.append(t)

    # ---------------- attention sublayer ----------------
    attn = tc_.alloc_tile_pool(name="attn", bufs=1)
    apsum = tc_.alloc_tile_pool(name="apsum", bufs=1, space="PSUM")

    wqk_sb = []
    for c in range(NC_CHUNKS):
        t = attn.tile([128, 2 * QKW], BF16, tag=f"wqk{c}", name=f"wqk{c}")
        nc.sync.dma_start(out=t[:], in_=wqk.ap()[ts(c, 128), :])
        wqk_sb.append(t)
    wv_sb = []
    for c in range(NC_CHUNKS):
        t = attn.tile([128, QKW], BF16, tag=f"wv{c}", name=f"wv{c}")
        nc.sync.dma_start(out=t[:], in_=wv.ap()[ts(c, 128), :])
        wv_sb.append(t)

    # q,k feature-major bf16 [128, T] x6 (first 3 = q chunks, last 3 = k chunks)
    qk_sb = [attn.tile([128, T], BF16, tag=f"qk{i}", name=f"qk{i}")
             for i in range(6)]
    # V token-major augmented with ones column: 16 tiles [128, 6*65] bf16
    vaug = [attn.tile([128, H_LOC * (HD + 1)], BF16, tag=f"va{i}", name=f"va{i}")
            for i in range(KT)]
    # normalized attention output, feature-major bf16 [384, T] as 3 tiles
    cvt_sb = [attn.tile([128, T], BF16, tag=f"cvt{i}", name=f"cvt{i}")
              for i in range(3)]

    # LN1 chains run two chunks ahead of the qkv GEMMs so the PE always has
    # stat sums or qkv matmuls ready while DVE normalize chains complete
    hps = {}

    def ln1_chain(tcix):
        sums = _ln_sums(nc, attn, apsum, x_tiles, ones_m, tcix, "l1",
                        "mmps", 3)
        mu_b, rstd_b = _ln_finish(nc, attn, eps_t, sums, "l1")
        hps[tcix] = _make_h(nc, attn, x_tiles, mu_b, rstd_b, tcix, "hp")

    ln1_chain(0)
    ln1_chain(1)

    for tcix in range(TC):
        tsl = ts(tcix, TN)
        if tcix + 2 < TC:
            ln1_chain(tcix + 2)
        hp = hps[tcix]
        # q,k feature-major
        for oc in range(6):
            ps = apsum.tile([128, TN], F32, tag="mmps", bufs=3, name="qkps")
            for c in range(NC_CHUNKS):
                nc.tensor.matmul(ps[:], wqk_sb[c][:, ts(oc, 128)], hp[c][:],
                                 start=(c == 0), stop=(c == NC_CHUNKS - 1))
            nc.vector.tensor_scalar_add(qk_sb[oc][:, tsl], ps[:],
                                        bqk_sb[:, oc:oc + 1])
        # V token-major (+ ones column)
        for s4 in range(TC):
            kt = tcix * 4 + s4
            vps = apsum.tile([128, QKW], F32, tag="mmps", bufs=3, name="vps")
            for c in range(NC_CHUNKS):
                nc.tensor.matmul(vps[:], hp[c][:, ts(s4, 128)], wv_sb[c][:],
                                 start=(c == 0), stop=(c == NC_CHUNKS - 1))
            va = vaug[kt]
            va_v = va[:].rearrange("p (h d) -> p h d", h=H_LOC)[:, :, 0:HD]
            nc.vector.tensor_copy(va_v, vps[:].rearrange("p (h d) -> p h d",
                                                         h=H_LOC))
            va_ones = va[:].rearrange("p (h d) -> p h d", h=H_LOC)[:, :, HD:HD + 1]
            nc.vector.memset(va_ones, 1.0)

    # attention proper + proj; big q chunks first so their AllReduce is in
    # flight the longest
    for qc in reversed(range(TC)):
        qsl = ts(qc, TN)
        for h in range(H_LOC):
            poff = (h % 2) * 64
            qh = qk_sb[h // 2][poff:poff + 64, qsl]
            khs = qk_sb[3 + h // 2]
            n_kc = 4 * (qc + 1)
            # S block: dense matmul stream; exp/mask chase on ACT/GpSimd
            pts = []
            for kc in range(n_kc):
                sps = apsum.tile([128, TN], F32, tag="sps", bufs=3, name="sps")
                nc.tensor.matmul(sps[:], khs[poff:poff + 64, ts(kc, 128)], qh,
                                 start=True, stop=True)
                pt = attn.tile([128, TN], BF16, tag="pt", bufs=14, name="pt")
                nc.scalar.activation(out=pt[:], in_=sps[:],
                                     func=mybir.ActivationFunctionType.Exp,
                                     scale=SCALE)
                j = kc - 4 * qc
                if j >= 0:
                    # causal band: zero cols < j*128, triangle in [j*128, +128)
                    w = j * 128 + 128
                    nc.gpsimd.affine_select(
                        out=pt[:, 0:w], in_=pt[:, 0:w],
                        pattern=[[1, w]],
                        compare_op=mybir.AluOpType.is_ge,
                        fill=0.0, base=-j * 128, channel_multiplier=-1)
                pts.append(pt)
            # PV block
            cvps = apsum.tile([HD + 1, TN], F32, tag="cvps", bufs=2, name="cvps")
            for kc in range(n_kc):
                nc.tensor.matmul(cvps[:], vaug[kc][:, ts(h, HD + 1)],
                                 pts[kc][:],
                                 start=(kc == 0), stop=(kc == n_kc - 1))
            # normalize by the ones-column sum
            rd = attn.tile([1, TN], F32, tag="rd", bufs=1, name="rd")
            nc.vector.tensor_copy(rd[:], cvps[HD:HD + 1, :])
            rd2 = attn.tile([1, TN], F32, tag="rd2", bufs=1, name="rd2")
            nc.vector.reciprocal_approx_fast(out=rd2[:], in_=rd[:])
            db = attn.tile([64, TN], F32, tag="db", bufs=1, name="db")
            nc.gpsimd.partition_broadcast(db[:], rd2[:])
            nc.vector.tensor_mul(cvt_sb[h // 2][poff:poff + 64, qsl],
                                 cvps[0:HD, :], db[:])
        # proj partials for this token chunk -> AR staging
        for oc in range(NC_CHUNKS):
            pps = apsum.tile([128, TN], F32, tag="mmps", bufs=3, name="pps")
            for c3 in range(3):
                nc.tensor.matmul(pps[:], wproj_sb[c3][:, ts(oc, 128)],
                                 cvt_sb[c3][:, qsl],
                                 start=(c3 == 0), stop=(c3 == 2))
            stg = attn.tile([128, TN], F32, tag="stg", bufs=2, name="stg")
            nc.vector.tensor_scalar_add(stg[:], pps[:], pbias_sb[:, oc:oc + 1])
            nc.sync.dma_start(out=ar1_in[qc][ts(oc, 128), :], in_=stg[:])
        nc.gpsimd.collective_compute(
            "AllReduce", mybir.AluOpType.add, replica_groups=REPLICA_GROUPS,
            ins=[ar1_in[qc].opt()], outs=[ar1_out[qc].opt()])

    attn.release()
    apsum.release()

    # ---------------- MLP sublayer ----------------
    mlp = tc_.alloc_tile_pool(name="mlp", bufs=1)
    mpsum = tc_.alloc_tile_pool(name="mpsum", bufs=1, space="PSUM")

    def residual1(tcix):
        tsl = ts(tcix, TN)
        for c in range(NC_CHUNKS):
            art = mlp.tile([128, TN], F32, tag="art", bufs=6, name="art")
            nc.sync.dma_start(out=art[:], in_=ar1_out[tcix][ts(c, 128), :])
            nc.vector.scalar_tensor_tensor(
                out=x_tiles[c][:, tsl], in0=art[:],
                scalar=bproj_sb[:, c:c + 1], in1=x_tiles[c][:, tsl],
                op0=mybir.AluOpType.add, op1=mybir.AluOpType.add)

    def make_h2(tcix):
        sums = _ln_sums(nc, mlp, mpsum, x_tiles, ones_m, tcix, "l2",
                        "fcps", 4)
        mu_b, rstd_b = _ln_finish(nc, mlp, eps_t, sums, "l2")
        return _make_h(nc, mlp, x_tiles, mu_b, rstd_b, tcix, "hp2")

    def fc_fc2_ar(tcix, hp):
        g_tiles = []
        for oc in range(12):
            ps = mpsum.tile([128, TN], F32, tag="fcps", bufs=4, name="fcps")
            for c in range(NC_CHUNKS):
                nc.tensor.matmul(ps[:], wfc_sb[c][:, ts(oc, 128)], hp[c][:],
                                 start=(c == 0), stop=(c == NC_CHUNKS - 1))
            g = mlp.tile([128, TN], BF16, tag="g", bufs=24, name="g")
            nc.scalar.activation(
                out=g[:], in_=ps[:],
                func=mybir.ActivationFunctionType.Gelu_apprx_tanh,
                bias=bfc_sb[:, oc:oc + 1])
            g_tiles.append(g)
        for oc in range(NC_CHUNKS):
            ps = mpsum.tile([128, TN], F32, tag="f2ps", bufs=4, name="f2ps")
            for c in range(12):
                nc.tensor.matmul(ps[:], wfc2_sb[c][:, ts(oc, 128)],
                                 g_tiles[c][:],
                                 start=(c == 0), stop=(c == 11))
            stg = mlp.tile([128, TN], F32, tag="stg2", bufs=2, name="stg2")
            nc.vector.tensor_copy(stg[:], ps[:])
            nc.sync.dma_start(out=ar2_in[tcix][ts(oc, 128), :], in_=stg[:])
        nc.gpsimd.collective_compute(
            "AllReduce", mybir.AluOpType.add, replica_groups=REPLICA_GROUPS,
            ins=[ar2_in[tcix].opt()], outs=[ar2_out[tcix].opt()])

    def residual2_store(tcix):
        tsl = ts(tcix, TN)
        for c in range(NC_CHUNKS):
            art = mlp.tile([128, TN], F32, tag="art2", bufs=3, name="art2")
            nc.sync.dma_start(out=art[:], in_=ar2_out[tcix][ts(c, 128), :])
            nc.vector.scalar_tensor_tensor(
                out=x_tiles[c][:, tsl], in0=art[:],
                scalar=bfc2_sb[:, c:c + 1], in1=x_tiles[c][:, tsl],
                op0=mybir.AluOpType.add, op1=mybir.AluOpType.add)
            nc.sync.dma_start(out=out_d.ap()[ts(c, 128), tsl],
                              in_=x_tiles[c][:, tsl])

    # tc3's AR1 fired first and is long done; tc0's AR1 is freshest, so its
    # residual is deferred until fc(3) keeps the PE busy
    h2 = {}
    for tcix in (3, 2, 1):
        residual1(tcix)
    for tcix in (3, 2, 1):
        h2[tcix] = make_h2(tcix)
    fc_fc2_ar(3, h2[3])
    residual1(0)
    h2[0] = make_h2(0)
    fc_fc2_ar(2, h2[2])
    fc_fc2_ar(1, h2[1])
    residual2_store(3)
    fc_fc2_ar(0, h2[0])
    residual2_store(2)
    residual2_store(1)
    residual2_store(0)

    mlp.release()
    mpsum.release()
    mlpw.release()
    persist.release()
    dram.release()


_NC_CACHE = None


def _get_nc():
    global _NC_CACHE
    if _NC_CACHE is None:
        _NC_CACHE = _build_nc()
    return _NC_CACHE


def _fold(v):
    return np.ascontiguousarray(v.reshape(-1, 128).T).astype(np.float32)


def _prep_core(core, x, ln1_g, ln1_b, w_attn, b_attn, w_proj, b_proj,
               ln2_g, ln2_b, w_fc, b_fc, w_fc2, b_fc2):
    b = core // 2
    tp = core % 2
    qs = slice(tp * QKW, (tp + 1) * QKW)
    ks = slice(C + tp * QKW, C + (tp + 1) * QKW)
    vs = slice(2 * C + tp * QKW, 2 * C + (tp + 1) * QKW)
    hs = slice(tp * HID_LOC, (tp + 1) * HID_LOC)

    x_fm = np.ascontiguousarray(x[b].T).astype(np.float32)

    wqk_h = np.concatenate([w_attn[:, qs], w_attn[:, ks]], axis=1)
    wqk_h = (wqk_h * ln1_g[:, None]).astype(np.float32)
    wv_h = (w_attn[:, vs] * ln1_g[:, None]).astype(np.float32)

    bqk = np.concatenate([b_attn[qs], b_attn[ks]]) + ln1_b @ np.concatenate(
        [w_attn[:, qs], w_attn[:, ks]], axis=1)
    bv = b_attn[vs] + ln1_b @ w_attn[:, vs]

    wproj_h = w_proj[tp * QKW:(tp + 1) * QKW, :]
    pbias = bv @ wproj_h                       # folded v-bias contribution
    wfc_h = (w_fc[:, hs] * ln2_g[:, None]).astype(np.float32)
    bfc = b_fc[hs] + ln2_b @ w_fc[:, hs]
    wfc2_h = w_fc2[hs, :]

    # b_proj / b_fc2 are added once per core after the AllReduce
    return {
        "x_fm": x_fm,
        "wqk": wqk_h.astype(ml_dtypes.bfloat16),
        "wv": wv_h.astype(ml_dtypes.bfloat16),
        "wproj": wproj_h.astype(ml_dtypes.bfloat16),
        "wfc": wfc_h.astype(ml_dtypes.bfloat16),
        "wfc2": wfc2_h.astype(ml_dtypes.bfloat16),
        "bqk": _fold(bqk),
        "pbias": _fold(pbias),
        "bproj": _fold(np.asarray(b_proj)),
        "bfc": _fold(np.asarray(b_fc)),
        "bfc2": _fold(np.asarray(b_fc2)),
    }


def kernel(x, ln1_g, ln1_b, w_attn, b_attn, w_proj, b_proj,
           ln2_g, ln2_b, w_fc, b_fc, w_fc2, b_fc2, _trace=False):
    args = [np.asarray(a, np.float32) for a in
            (x, ln1_g, ln1_b, w_attn, b_attn, w_proj, b_proj,
             ln2_g, ln2_b, w_fc, b_fc, w_fc2, b_fc2)]
    nc = _get_nc()
    in_maps = [_prep_core(core, *args) for core in range(N_CORES)]
    res = run_bass_kernel_spmd(nc, in_maps, list(range(N_CORES)),
                               trace=_trace)
    out = np.empty((B, T, C), np.float32)
    for b in range(B):
        out[b] = res.results[2 * b]["out_fm"].T
    kernel._last_result = res
    return out
